# revision 17
# baseline (speedup 1.0000x reference)
"""LucidLinearAttention Trainium2 kernel (8-core SPMD), bf16 dataflow.

Sharding: batch b = core//2 (4 batches), head-group hg = core%2 (8 heads each).
Each core: qkv projection for its heads, chunked linear attention with a
hybrid block-causal formulation (256-col attention blocks, 64-wide buckets,
exclusive block carry C in f32), partial output projection. Host sums the two
head-group partials per batch.

Head-pair (h2) stacking keeps every projection / out-projection matmul at
M=K=128. S^T is computed only over the causally visible column range; the
bucket mask is realized by copying rect+wedge regions into pre-zeroed SBUF
tiles. All matmul operands are bf16 (predicted rel err ~6e-3 vs 2e-2 gate);
PSUM accumulation stays f32.

Scheduling: the PE stream is software-pipelined at two levels. The
out-projection of attention block a-1 and the Q/K/V projection + K^T
transposes of projection block N+1 are emitted as "filler" between the OUT
iterations of the current attention block, so the PE never idles on the
DVE/ACT normalization chain (recip -> dinv broadcast -> xot multiply).
"""
import sys
import numpy as np

for p in ("/opt/trn_rl_repo", "/root/.axon_site/_ro/trn_rl_repo"):
    if p not in sys.path:
        sys.path.insert(0, p)

import ml_dtypes
import concourse.mybir as mybir
import concourse.tile as tile
from concourse import bacc
from concourse.bass_utils import run_bass_kernel_spmd
from concourse.masks import make_identity

F32 = mybir.dt.float32
F32R = mybir.dt.float32r
BF16 = mybir.dt.bfloat16
EXP = mybir.ActivationFunctionType.Exp

B, T, D = 4, 4096, 1024
NH, HD, BUCKET = 16, 64, 64
HPC = 8                 # heads per core
GD = HPC * HD           # 512
NPB = 8                 # projection blocks
PBT = T // NPB          # 512 cols
NAB = 16                # attention blocks
ABT = T // NAB          # 256 cols
NC_CORES = 8

_CACHE = {}
_BF = ml_dtypes.bfloat16


def _build():
    nc = bacc.Bacc("TRN2", target_bir_lowering=False, debug=False,
                   num_devices=NC_CORES)
    xT = nc.dram_tensor("xT", [D, T], BF16, kind="ExternalInput").ap()
    wqT = nc.dram_tensor("wqT", [D, GD], BF16, kind="ExternalInput").ap()
    wkT = nc.dram_tensor("wkT", [D, GD], BF16, kind="ExternalInput").ap()
    wvT = nc.dram_tensor("wvT", [D, GD], BF16, kind="ExternalInput").ap()
    woT = nc.dram_tensor("woT", [GD, D], BF16, kind="ExternalInput").ap()
    y = nc.dram_tensor("y", [T, D], F32, kind="ExternalOutput").ap()

    with tile.TileContext(nc) as tc:
        with nc.allow_low_precision(reason="bf16 matmul dataflow by design"), \
             tc.tile_pool(name="w", bufs=1) as wp, \
             tc.tile_pool(name="per", bufs=1) as pp, \
             tc.tile_pool(name="sb", bufs=1) as sbp, \
             tc.tile_pool(name="ps", bufs=1, space="PSUM") as ps:

            # ---- resident weights (direct bf16 DMA) ---------------------
            wq_sb = [wp.tile([128, GD], BF16, tag=f"wq{dc}", name=f"wq{dc}") for dc in range(8)]
            wk_sb = [wp.tile([128, GD], BF16, tag=f"wk{dc}", name=f"wk{dc}") for dc in range(8)]
            wv_sb = [wp.tile([128, GD], BF16, tag=f"wv{dc}", name=f"wv{dc}") for dc in range(8)]
            wo_sb = [wp.tile([128, D], BF16, tag=f"wo{h2}", name=f"wo{h2}") for h2 in range(4)]

            xtiles = {}

            def load_x(pb0):
                xs = [sbp.tile([128, 2 * PBT], BF16, tag=f"xsb{dc}", name=f"xsb{dc}", bufs=2)
                      for dc in range(8)]
                for dc in range(8):
                    nc.sync.dma_start(
                        xs[dc][:], xT[128 * dc:128 * (dc + 1), PBT * pb0:PBT * (pb0 + 2)])
                xtiles[pb0] = xs
                xtiles[pb0 + 1] = xs

            for dc in range(8):
                nc.sync.dma_start(wk_sb[dc][:], wkT[128 * dc:128 * (dc + 1), :])
            load_x(0)

            # ---- persistent state --------------------------------------
            ident_f = pp.tile([128, 128], F32, tag="ident_f")
            make_identity(nc, ident_f[:])
            ident_b = pp.tile([128, 128], BF16, tag="ident_b")
            nc.vector.tensor_copy(ident_b[:], ident_f[:])
            # dinv broadcast weights: top half / bottom half of a head pair
            bv_f = pp.tile([1, 256], F32, tag="bv_f")
            nc.vector.memset(bv_f[:], 0.0)
            nc.vector.memset(bv_f[0:1, 0:64], 1.0)
            nc.vector.memset(bv_f[0:1, 192:256], 1.0)
            bvt = pp.tile([1, 128], F32R, tag="bvt")
            nc.vector.tensor_copy(bvt[:], bv_f[0:1, 0:128])
            bvb = pp.tile([1, 128], F32R, tag="bvb")
            nc.vector.tensor_copy(bvb[:], bv_f[0:1, 128:256])
            # caug[h]: [128, 66] bf16; even h data in rows 0:64, odd in 64:128,
            # other half stays zero (K=128 padding for the inter matmul).
            caug = [pp.tile([128, 66], BF16, tag=f"caug{h}", name=f"caug{h}") for h in range(HPC)]
            for h in range(HPC):
                nc.vector.memset(caug[h][:], 0.0)
            caug_f32 = [pp.tile([64, 66], F32, tag=f"caugf{h}", name=f"caugf{h}") for h in range(HPC)]
            for h in range(HPC):
                nc.vector.memset(caug_f32[h][:], 0.0)
            # vaug[par][c]: [128, 8*66] bf16; per head slot: [v(64) | 1 | 0]
            vaug = [[pp.tile([128, HPC * 66], BF16, tag=f"vaug{par}_{c}", name=f"vaug{par}_{c}")
                     for c in range(4)] for par in range(2)]
            for par in range(2):
                for c in range(4):
                    nc.vector.memset(vaug[par][c][:], 0.0)
                    for h in range(HPC):
                        nc.vector.memset(vaug[par][c][:, 66 * h + 64:66 * h + 65], 1.0)
            # ssb[h][par]: [128, 320] bf16 masked S^T; cols 0:256 chunk0
            # (queries 0:256), cols 256:320 chunk1 wedge (queries 192:256).
            # Zero strips preset once.
            ssb = [[pp.tile([128, 320], BF16, tag=f"ssb{h}_{s}", name=f"ssb{h}_{s}")
                    for s in range(2)] for h in range(HPC)]
            for h in range(HPC):
                for s in range(2):
                    nc.vector.memset(ssb[h][s][:], 0.0)

            # ---- projection-work fillers -------------------------------
            def proj_filler(kind, idx, pb2, store):
                par2 = pb2 % 2
                xoff = PBT * par2

                def emit():
                    xs = xtiles[pb2]
                    if kind == "q":
                        pq = ps.tile([128, PBT], F32, tag="proj", name="pq", bufs=2)
                        for dc in range(8):
                            nc.tensor.matmul(
                                pq[:], wq_sb[dc][:, 128 * idx:128 * (idx + 1)],
                                xs[dc][:, xoff:xoff + PBT],
                                start=(dc == 0), stop=(dc == 7))
                        qt = sbp.tile([128, PBT], BF16, tag=f"qtu{idx}",
                                      name=f"qtu{idx}", bufs=2)
                        nc.scalar.activation(qt[:], pq[:], EXP)
                        store["qtu2"][idx] = qt
                    elif kind == "k":
                        pk = ps.tile([128, GD], F32, tag="proj", name="pk", bufs=2)
                        for dc in range(8):
                            nc.tensor.matmul(
                                pk[:], xs[dc][:, xoff + 128 * idx:xoff + 128 * (idx + 1)],
                                wk_sb[dc][:],
                                start=(dc == 0), stop=(dc == 7))
                        kt_ = sbp.tile([128, GD], BF16, tag=f"ksb{idx}",
                                       name=f"ksb{idx}", bufs=2)
                        nc.scalar.activation(kt_[:], pk[:], EXP)
                        store["ksb"][idx] = kt_
                    elif kind == "v":
                        pv = ps.tile([128, GD], F32, tag="proj", name="pv", bufs=2)
                        for dc in range(8):
                            nc.tensor.matmul(
                                pv[:], xs[dc][:, xoff + 128 * idx:xoff + 128 * (idx + 1)],
                                wv_sb[dc][:],
                                start=(dc == 0), stop=(dc == 7))
                        vv = vaug[par2][idx][:].rearrange("p (h c) -> p h c", c=66)
                        pvv = pv[:].rearrange("p (h c) -> p h c", c=64)
                        nc.vector.tensor_copy(vv[:, :, 0:64], pvv[:, :, :])
                    else:  # "t": K^T transpose for head pair idx
                        ktp = ps.tile([128, PBT], BF16, tag="proj", name="ktp", bufs=2)
                        for c in range(4):
                            nc.tensor.transpose(
                                ktp[:, 128 * c:128 * (c + 1)],
                                store["ksb"][c][:, 128 * idx:128 * (idx + 1)], ident_b[:])
                        kt = sbp.tile([128, PBT], BF16, tag=f"kt2{idx}",
                                      name=f"kt2{idx}", bufs=2)
                        nc.vector.tensor_copy(kt[:], ktp[:])
                        store["kt2"][idx] = kt
                return emit

            ORDER = ([("k", c) for c in range(4)]
                     + [("q", 0), ("q", 1), ("q", 2), ("q", 3)]
                     + [("v", 0), ("v", 1), ("v", 2), ("v", 3)]
                     + [("t", h2) for h2 in range(4)])

            def make_fillers(pb2, store):
                return [proj_filler(kind, idx, pb2, store) for kind, idx in ORDER]

            # prologue: block 0 projections emitted up front
            cur = {"qtu2": [None] * 4, "ksb": [None] * 4, "kt2": [None] * 4}
            for fi, f in enumerate(make_fillers(0, cur)):
                if fi == 1:
                    for dc in range(8):
                        nc.sync.dma_start(wq_sb[dc][:], wqT[128 * dc:128 * (dc + 1), :])
                if fi == 3:
                    for dc in range(8):
                        nc.sync.dma_start(wv_sb[dc][:], wvT[128 * dc:128 * (dc + 1), :])
                    load_x(2)
                if fi == 7:
                    for h2 in range(4):
                        nc.sync.dma_start(wo_sb[h2][:], woT[128 * h2:128 * (h2 + 1), :])
                f()

            pending = []  # out-projection work deferred one attn block

            # ---- main loop over attention blocks -----------------------
            for pb in range(NPB):
                par = pb % 2
                if pb >= 2 and par == 0 and pb + 2 < NPB:
                    load_x(pb + 2)

                nxt = {"qtu2": [None] * 4, "ksb": [None] * 4, "kt2": [None] * 4}
                fillers = make_fillers(pb + 1, nxt) if pb + 1 < NPB else []
                qtu2, ksb, kt2 = cur["qtu2"], cur["ksb"], cur["kt2"]

                for ab in range(2):
                    a = 2 * pb + ab
                    qoff = ABT * ab
                    spar = a % 2
                    fq = fillers[8 * ab:8 * ab + 8]
                    xot2 = [sbp.tile([128, ABT], BF16, tag=f"xot{h2}",
                                     name=f"xot{h2}", bufs=2) for h2 in range(4)]

                    # S^T (visible range only) + masked rect/wedge copies
                    for h in range(HPC):
                        h2, hb = h // 2, (h % 2) * 64
                        pst = ps.tile([128, ABT], F32, tag="spb", name="pst", bufs=3)
                        nc.tensor.matmul(
                            pst[:, 0:192],
                            kt2[h2][hb:hb + 64, qoff:qoff + 128],
                            qtu2[h2][hb:hb + 64, qoff + 64:qoff + 256],
                            start=True, stop=True)
                        nc.tensor.matmul(
                            pst[:, 192:256],
                            kt2[h2][hb:hb + 64, qoff + 128:qoff + 256],
                            qtu2[h2][hb:hb + 64, qoff + 192:qoff + 256],
                            start=True, stop=True)
                        sb_t = ssb[h][spar]
                        nc.vector.tensor_copy(sb_t[:, 128:256], pst[:, 64:192])
                        wsrc = pst[0:64, :].rearrange("p (a b) -> p a b", b=64)
                        wdst = sb_t[0:64, 64:320].rearrange("p (a b) -> p a b", b=64)
                        nc.scalar.copy(wdst[:, 0:4:3, :], wsrc[:, 0:4:3, :])
                        if h == 2 and fq:
                            fq[0]()
                        elif h == 5 and fq:
                            fq[1]()

                    # OUT groups + normalization, with out-projection of the
                    # previous attn block and next-block projection work
                    # interleaved as PE filler.
                    opbs, dvps = [None] * HPC, [None] * HPC
                    ow = pending.pop() if pending else None
                    ysb_cur = [None]

                    def norm_pair(p):
                        opbE = opbs[2 * p]
                        nc.tensor.matmul(opbE[0:128, 256:512], bvt[:, :],
                                         dvps[2 * p][:], start=True, stop=False)
                        nc.tensor.matmul(opbE[0:128, 256:512], bvb[:, :],
                                         dvps[2 * p + 1][:], start=False, stop=True)
                        xu = sbp.tile([128, ABT], BF16, tag="xotu", name="xotu", bufs=3)
                        nc.scalar.copy(xu[0:64, :], opbE[0:64, 0:256])
                        nc.scalar.copy(xu[64:128, :], opbs[2 * p + 1][0:64, 0:256])
                        nc.vector.tensor_mul(
                            xot2[p][:], opbE[:, 256:512], xu[:])

                    def emit_py_group(g):
                        xot2_p, a_p = ow
                        tch, fc = g // 2, g % 2
                        if fc == 0:
                            ysb_cur[0] = sbp.tile([128, D], F32, tag="ysb",
                                                  name="ysb", bufs=3)
                        py = ps.tile([128, GD], F32, tag="proj", name="py", bufs=2)
                        for h2p in range(4):
                            nc.tensor.matmul(
                                py[:],
                                xot2_p[h2p][:, 128 * tch:128 * (tch + 1)],
                                wo_sb[h2p][:, GD * fc:GD * (fc + 1)],
                                start=(h2p == 0), stop=(h2p == 3))
                        nc.scalar.copy(ysb_cur[0][:, GD * fc:GD * (fc + 1)], py[:])
                        if fc == 1:
                            r0 = ABT * a_p + 128 * tch
                            nc.sync.dma_start(y[r0:r0 + 128, :], ysb_cur[0][:])

                    pcMs = [None] * 4
                    for h in range(HPC):
                        h2, hb = h // 2, (h % 2) * 64
                        opb = ps.tile([128, 512], F32, tag="opb", name="opb", bufs=3)
                        opbs[h] = opb
                        nc.tensor.matmul(opb[0:66, 0:256], caug[h][:],
                                         qtu2[h2][:, qoff:qoff + 256],
                                         start=True, stop=False)
                        nc.tensor.matmul(opb[0:66, 64:256],
                                         vaug[par][2 * ab][:, 66 * h:66 * h + 66],
                                         ssb[h][spar][:, 64:256],
                                         start=False, stop=False)
                        nc.tensor.matmul(opb[0:66, 192:256],
                                         vaug[par][2 * ab + 1][:, 66 * h:66 * h + 66],
                                         ssb[h][spar][:, 256:320],
                                         start=False, stop=True)
                        dv = sbp.tile([1, ABT], F32R, tag="dv", name="dv", bufs=6)
                        dvps[h] = dv
                        if a == 0:
                            nc.vector.tensor_scalar_max(dv[:], opb[64:65, 0:256], 1e-30)
                            nc.vector.reciprocal(dv[:], dv[:])
                        else:
                            nc.vector.reciprocal(dv[:], opb[64:65, 0:256])
                        # C / kcum update, pair-merged: one [128,132] group
                        # covers both heads; adds read the diagonal blocks.
                        if h % 2 == 0:
                            pcM = ps.tile([128, 132], F32, tag="spb", name="pcM", bufs=3)
                            pcMs[h // 2] = pcM
                            for ci in range(2):
                                c = 2 * ab + ci
                                nc.tensor.matmul(
                                    pcM[:], ksb[c][:, 64 * h:64 * h + 128],
                                    vaug[par][c][:, 66 * h:66 * h + 132],
                                    start=(ci == 0), stop=(ci == 1))
                            nc.vector.tensor_add(caug_f32[h][:], caug_f32[h][:],
                                                 pcM[0:64, 0:66])
                        else:
                            pcM = pcMs[h // 2]
                            nc.vector.tensor_add(caug_f32[h][:], caug_f32[h][:],
                                                 pcM[64:128, 66:132])
                        nc.gpsimd.tensor_copy(caug[h][hb:hb + 64, :], caug_f32[h][:])
                        if h + 2 < len(fq):
                            fq[h + 2]()
                        if ow is not None and h % 2 == 1 and h >= 3:
                            emit_py_group((h - 3) // 2)
                        if h % 2 == 0 and h >= 2:
                            norm_pair((h - 2) // 2)
                    norm_pair(3)
                    if ow is not None:
                        emit_py_group(3)

                    pending.append((xot2, a))

                cur = nxt

            # epilogue: last attention block's out-projection
            ow = pending.pop()
            ysb_cur = [None]
            for g in range(4):
                emit_py_group(g)

    nc.compile()
    return nc


def _get_nc():
    if "nc" not in _CACHE:
        _CACHE["nc"] = _build()
    return _CACHE["nc"]


def kernel(x, W_qkv, W_out):
    x = np.asarray(x, dtype=np.float32)
    W_qkv = np.asarray(W_qkv, dtype=np.float32)
    W_out = np.asarray(W_out, dtype=np.float32)
    nc = _get_nc()

    xTs = [np.ascontiguousarray(x[b].T).astype(_BF) for b in range(B)]
    in_maps = []
    for c in range(NC_CORES):
        b, hg = c // 2, c % 2
        s = slice(hg * GD, (hg + 1) * GD)
        in_maps.append({
            "xT": xTs[b],
            "wqT": np.ascontiguousarray(W_qkv[0 * D:1 * D][s].T).astype(_BF),
            "wkT": np.ascontiguousarray(W_qkv[1 * D:2 * D][s].T).astype(_BF),
            "wvT": np.ascontiguousarray(W_qkv[2 * D:3 * D][s].T).astype(_BF),
            "woT": np.ascontiguousarray(W_out[:, s].T).astype(_BF),
        })
    res = run_bass_kernel_spmd(nc, in_maps, core_ids=list(range(NC_CORES)))
    out = np.empty((B, T, D), dtype=np.float32)
    for b in range(B):
        out[b] = res.results[2 * b]["y"] + res.results[2 * b + 1]["y"]
    return out
